# revision 9
# baseline (speedup 1.0000x reference)
"""FourierLinear Trainium2 kernel v4 — host-precomputed trig tables.

v3 generated cos/sin tables on device (DVE int ops + ScalarE Sin +
GpSimd adds), which kept Vector/Scalar/GpSimd ~50% busy and stalled the
PE (60% busy, 1.0 ms measured).  The tables are x-independent constants,
so v4 precomputes them on the host and streams them from HBM, leaving
the device a pure two-stage fp16 matmul pipeline:

  stage 1 (k-parity fold):  u_f[m] = sum_{k'<2048} trig(w a_f k') xfold
    with xfold = x_lo +- x_hi picked by parity(a_f)
      -> psum f32, copied to SBUF as vc/vs = u * 2^-8 (fp16)
  stage 2 (l-parity fold):  yE/yO accumulate s_f 2^-8 (uc cos - us sin)
    over even-b / odd-b frequencies;  y[l'] = yE+yO, y[l'+2048] = yE-yO

The 2^-16 ifft2 norm (* 256 scale) is split 2^-8 at the stage-1 copy
and 2^-8 folded into the stage-2 tables so every fp16 tensor stays in
the normal range.  Frequencies are grouped by (a%2, b%2) and padded to
128-chunks exactly as in v3.

Per-core upload: 8 MB folded x + 37.8 MB tables (all fp16).  In-kernel
HBM reads ~77 GB/s per stage — far under the DMA roofline.  PE work:
2 stages x FC2*16*2 (or 4*FC2*8*2) matmuls of 512 free = ~1.18M cycles
~ 490 us at 2.4 GHz.
"""

import numpy as np

import concourse.mybir as mybir
import concourse.tile as tile
from concourse import bacc
from concourse.bass_utils import run_bass_kernel_spmd

N_CORES = 8
IN_F = 4096
OUT_F = 4096
NF = 2048
ROWS = 8192
M = ROWS // N_CORES   # 1024 rows per core
P = 128
KH = IN_F // 2        # 2048 folded k' range
KCH = KH // P         # 16 k'-chunks
LH = OUT_F // 2       # 2048 folded l' range
NT = 512
LTH = LH // NT        # 4 l'-tiles
MS = M // P           # 8 row blocks

LAST_RESULTS = None
_NC_CACHE = None


def _build_nc(gcs):
    # gcs: chunks per (a%2, b%2) parity group; group g owns f-chunks
    # [off[g], off[g+1]) and uses xp (g<2) / xm (g>=2) in stage 1,
    # yE (g in {0,2}) / yO (g in {1,3}) in stage 2.
    off = [0]
    for g in range(4):
        off.append(off[-1] + gcs[g])
    FC2 = off[4]
    chunk_grp = []
    for g in range(4):
        chunk_grp += [g] * gcs[g]
    f32 = mybir.dt.float32
    f16 = mybir.dt.float16
    mult = mybir.AluOpType.mult
    add = mybir.AluOpType.add
    sub = mybir.AluOpType.subtract

    nc = bacc.Bacc(None)
    xpT = nc.declare_dram_parameter("xpT", [KH, M], f16, isOutput=False)
    xmT = nc.declare_dram_parameter("xmT", [KH, M], f16, isOutput=False)
    t1c = nc.declare_dram_parameter("t1c", [FC2 * KCH * P, P], f16, isOutput=False)
    t1s = nc.declare_dram_parameter("t1s", [FC2 * KCH * P, P], f16, isOutput=False)
    t2c = nc.declare_dram_parameter("t2c", [FC2 * LTH * P, NT], f16, isOutput=False)
    t2s = nc.declare_dram_parameter("t2s", [FC2 * LTH * P, NT], f16, isOutput=False)
    out = nc.declare_dram_parameter("out", [M, OUT_F], f16, isOutput=True)

    xpp = xpT[:].rearrange("(kc p) m -> p kc m", p=P)
    xmp = xmT[:].rearrange("(kc p) m -> p kc m", p=P)
    t1cp = t1c[:].rearrange("(fb kc p) j -> p fb kc j", fb=FC2, kc=KCH, p=P)
    t1sp = t1s[:].rearrange("(fb kc p) j -> p fb kc j", fb=FC2, kc=KCH, p=P)
    t2cp = t2c[:].rearrange("(fc lt p) l -> p fc lt l", fc=FC2, lt=LTH, p=P)
    t2sp = t2s[:].rearrange("(fc lt p) l -> p fc lt l", fc=FC2, lt=LTH, p=P)
    outp = out[:].rearrange("(ms p) n -> p ms n", p=P)

    with tile.TileContext(nc) as tc:
        with (
            tc.tile_pool(name="v", bufs=1) as vpool,
            tc.tile_pool(name="o", bufs=4) as opool,
        ):
            vc = vpool.tile([P, FC2, M], f16)   # u_cos * 2^-8, 36 KB/part
            vs = vpool.tile([P, FC2, M], f16)

            # ---- stage 1: u^T[f,m] = T1^T @ xfold -> vc/vs
            with (
                tc.tile_pool(name="x", bufs=1) as xpool,
                tc.tile_pool(name="t1", bufs=3) as t1p,
                tc.tile_pool(name="ps1", bufs=4, space="PSUM") as ps1,
            ):
                xp = xpool.tile([P, KCH, M], f16)   # 32 KB/part
                xm = xpool.tile([P, KCH, M], f16)
                # First fb's tables go out first so the PE can start ~4us in;
                # x streams per-kc chunk behind them (xp feeds fb 0.., xm is
                # not needed until the a-odd groups halfway through stage 1).
                tcb0 = t1p.tile([P, KCH, P], f16, tag="tc")
                tsb0 = t1p.tile([P, KCH, P], f16, tag="ts")
                nc.sync.dma_start(tcb0[:], t1cp[:, 0, :, :])
                nc.scalar.dma_start(tsb0[:], t1sp[:, 0, :, :])
                for kc in range(KCH):
                    eng = nc.gpsimd if kc % 2 == 0 else nc.sync
                    eng.dma_start(xp[:, kc, :], xpp[:, kc, :])
                for kc in range(KCH):
                    eng = nc.gpsimd if kc % 2 == 0 else nc.sync
                    eng.dma_start(xm[:, kc, :], xmp[:, kc, :])
                for fb in range(FC2):
                    xf = xp if chunk_grp[fb] < 2 else xm
                    if fb == 0:
                        tcb, tsb = tcb0, tsb0
                    else:
                        tcb = t1p.tile([P, KCH, P], f16, tag="tc")
                        tsb = t1p.tile([P, KCH, P], f16, tag="ts")
                        nc.sync.dma_start(tcb[:], t1cp[:, fb, :, :])
                        nc.scalar.dma_start(tsb[:], t1sp[:, fb, :, :])
                    psc = ps1.tile([P, M], f32, tag="u", name=f"psc{fb}")
                    pss = ps1.tile([P, M], f32, tag="u", name=f"pss{fb}")
                    for kc in range(KCH):
                        st, sp = kc == 0, kc == KCH - 1
                        nc.tensor.matmul(psc[:, 0:NT], tcb[:, kc, :],
                                         xf[:, kc, 0:NT], start=st, stop=sp)
                        nc.tensor.matmul(psc[:, NT:M], tcb[:, kc, :],
                                         xf[:, kc, NT:M], start=st, stop=sp)
                        nc.tensor.matmul(pss[:, 0:NT], tsb[:, kc, :],
                                         xf[:, kc, 0:NT], start=st, stop=sp)
                        nc.tensor.matmul(pss[:, NT:M], tsb[:, kc, :],
                                         xf[:, kc, NT:M], start=st, stop=sp)
                    nc.scalar.mul(vc[:, fb, :], psc[:], 2.0 ** -8)
                    nc.vector.tensor_scalar(vs[:, fb, :], pss[:], 2.0 ** -8,
                                            None, mult)

            # ---- stage 2: yE/yO over l' < 2048, y = yE +- yO
            # E accumulates in all 8 PSUM banks, parks in SBUF, then O
            # reuses the banks; combine yE +- psO.  The minus sign of the
            # sin term is folded into t2s on the host.
            e_chunks = [fc for fc in range(FC2) if chunk_grp[fc] % 2 == 0]
            o_chunks = [fc for fc in range(FC2) if chunk_grp[fc] % 2 == 1]
            with (
                tc.tile_pool(name="t2", bufs=8) as t2p,
                tc.tile_pool(name="ye", bufs=1) as yep,
                tc.tile_pool(name="ps2", bufs=8, space="PSUM") as ps2,
            ):
                def run_chunks(chunks, pstag):
                    pss = [ps2.tile([P, NT], f32, tag="y", name=f"{pstag}{ms}")
                           for ms in range(MS)]
                    for fc in chunks:
                        bc = t2p.tile([P, NT], f16, tag="bc")
                        bs = t2p.tile([P, NT], f16, tag="bs")
                        nc.sync.dma_start(bc[:], t2cp[:, fc, lt, :])
                        nc.gpsimd.dma_start(bs[:], t2sp[:, fc, lt, :])
                        for ms in range(MS):
                            nc.tensor.matmul(
                                pss[ms][:], vc[:, fc, ms * P : (ms + 1) * P],
                                bc[:], start=(fc == chunks[0]), stop=False)
                            nc.tensor.matmul(
                                pss[ms][:], vs[:, fc, ms * P : (ms + 1) * P],
                                bs[:], start=False, stop=(fc == chunks[-1]))
                    return pss

                for lt in range(LTH):
                    yE = yep.tile([P, MS, NT], f32, tag="ye", name=f"yE{lt}")
                    psA = run_chunks(e_chunks, f"psA{lt}_")
                    for ms in range(MS):
                        if ms % 2 == 0:
                            nc.scalar.copy(out=yE[:, ms, :], in_=psA[ms][:])
                        else:
                            nc.vector.tensor_copy(out=yE[:, ms, :],
                                                  in_=psA[ms][:])
                    psB = run_chunks(o_chunks, f"psB{lt}_")
                    for ms in range(MS):
                        olo = opool.tile([P, NT], f16, tag="olo", name="olo")
                        ohi = opool.tile([P, NT], f16, tag="ohi", name="ohi")
                        nc.vector.tensor_tensor(out=olo[:], in0=yE[:, ms, :],
                                                in1=psB[ms][:], op=add)
                        nc.vector.tensor_tensor(out=ohi[:], in0=yE[:, ms, :],
                                                in1=psB[ms][:], op=sub)
                        nc.scalar.dma_start(
                            outp[:, ms, lt * NT : (lt + 1) * NT], olo[:])
                        nc.sync.dma_start(
                            outp[:, ms, LH + lt * NT : LH + (lt + 1) * NT],
                            ohi[:])
    nc.finalize()
    return nc


def _host_prep(x, spectrum, indices):
    x2 = np.asarray(x, dtype=np.float32).reshape(ROWS, IN_F)
    idx = np.asarray(indices, dtype=np.int64)
    s = np.asarray(spectrum, dtype=np.float32)
    a, b = idx[0], idx[1]

    # reference scatter is last-write-wins on duplicate (a,b) pairs
    keys = a * OUT_F + b
    _, first_of_reversed = np.unique(keys[::-1], return_index=True)
    keep = np.zeros(NF, dtype=bool)
    keep[NF - 1 - first_of_reversed] = True
    s_eff = np.where(keep, s, 0.0).astype(np.float32)

    # group frequencies by (a%2, b%2); pad each group to a whole number of
    # 128-chunks with zero-spectrum dummies whose indices keep the parity
    sels = [np.nonzero(((a % 2) == (g >> 1)) & ((b % 2) == (g & 1)))[0]
            for g in range(4)]
    gcs = [max(1, -(-len(sel) // P)) for sel in sels]   # chunks per group, >= 1
    FC2 = sum(gcs)
    NF2 = FC2 * P
    a2 = np.zeros(NF2, np.int64)
    b2 = np.zeros(NF2, np.int64)
    s2 = np.zeros(NF2, np.float32)
    o = 0
    for g in range(4):
        sel = sels[g]
        a2[o : o + len(sel)] = a[sel]
        b2[o : o + len(sel)] = b[sel]
        s2[o : o + len(sel)] = s_eff[sel]
        a2[o + len(sel) : o + gcs[g] * P] = g >> 1
        b2[o + len(sel) : o + gcs[g] * P] = g & 1
        o += gcs[g] * P

    w = 2.0 * np.pi / 4096.0
    kk = np.arange(KH)
    ll = np.arange(LH)
    # stage 1 tables [k', f] -> [fb, kc, p(k'), j(f)]
    ph1 = (a2[None, :] * kk[:, None]) % 4096
    t1c_full = np.cos(w * ph1, dtype=np.float32)
    t1s_full = np.sin(w * ph1, dtype=np.float32)
    t1c = np.ascontiguousarray(
        t1c_full.reshape(KCH, P, FC2, P).transpose(2, 0, 1, 3)
        .reshape(FC2 * KCH * P, P).astype(np.float16))
    t1s = np.ascontiguousarray(
        t1s_full.reshape(KCH, P, FC2, P).transpose(2, 0, 1, 3)
        .reshape(FC2 * KCH * P, P).astype(np.float16))
    # stage 2 tables [f, l'] with s*2^-8 folded in (sin term negated so the
    # psum accumulation is a pure add) -> [fc, lt, p(f), l']
    ph2 = (b2[:, None] * ll[None, :]) % 4096
    sc = (s2 * 2.0 ** -8)[:, None]
    t2c_full = np.cos(w * ph2, dtype=np.float32) * sc
    t2s_full = np.sin(w * ph2, dtype=np.float32) * (-sc)
    t2c = np.ascontiguousarray(
        t2c_full.reshape(FC2, P, LTH, NT).transpose(0, 2, 1, 3)
        .reshape(FC2 * LTH * P, NT).astype(np.float16))
    t2s = np.ascontiguousarray(
        t2s_full.reshape(FC2, P, LTH, NT).transpose(0, 2, 1, 3)
        .reshape(FC2 * LTH * P, NT).astype(np.float16))

    xp16 = (x2[:, :KH] + x2[:, KH:]).astype(np.float16)
    xm16 = (x2[:, :KH] - x2[:, KH:]).astype(np.float16)
    return xp16, xm16, t1c, t1s, t2c, t2s, tuple(gcs)


def kernel(x, spectrum, indices):
    global _NC_CACHE, LAST_RESULTS
    xp16, xm16, t1c, t1s, t2c, t2s, gcs = _host_prep(x, spectrum, indices)

    if _NC_CACHE is None or _NC_CACHE[0] != gcs:
        _NC_CACHE = (gcs, _build_nc(gcs))
    nc = _NC_CACHE[1]

    in_maps = [
        {
            "xpT": np.ascontiguousarray(xp16[j * M : (j + 1) * M].T),
            "xmT": np.ascontiguousarray(xm16[j * M : (j + 1) * M].T),
            "t1c": t1c,
            "t1s": t1s,
            "t2c": t2c,
            "t2s": t2s,
        }
        for j in range(N_CORES)
    ]
    res = run_bass_kernel_spmd(nc, in_maps, list(range(N_CORES)))
    LAST_RESULTS = res
    out = np.concatenate(
        [res.results[j]["out"].astype(np.float32) for j in range(N_CORES)], axis=0
    )
    return out.reshape(np.asarray(x).shape[:-1] + (OUT_F,))


# revision 11
# speedup vs baseline: 1.0233x; 1.0233x over previous
"""FourierLinear Trainium2 kernel v4 — host-precomputed trig tables.

v3 generated cos/sin tables on device (DVE int ops + ScalarE Sin +
GpSimd adds), which kept Vector/Scalar/GpSimd ~50% busy and stalled the
PE (60% busy, 1.0 ms measured).  The tables are x-independent constants,
so v4 precomputes them on the host and streams them from HBM, leaving
the device a pure two-stage fp16 matmul pipeline:

  stage 1 (k-parity fold):  u_f[m] = sum_{k'<2048} trig(w a_f k') xfold
    with xfold = x_lo +- x_hi picked by parity(a_f)
      -> psum f32, copied to SBUF as vc/vs = u * 2^-8 (fp16)
  stage 2 (l-parity fold):  yE/yO accumulate s_f 2^-8 (uc cos - us sin)
    over even-b / odd-b frequencies;  y[l'] = yE+yO, y[l'+2048] = yE-yO

The 2^-16 ifft2 norm (* 256 scale) is split 2^-8 at the stage-1 copy
and 2^-8 folded into the stage-2 tables so every fp16 tensor stays in
the normal range.  Frequencies are grouped by (a%2, b%2) and padded to
128-chunks exactly as in v3.

Per-core upload: 8 MB folded x + 37.8 MB tables (all fp16).  In-kernel
HBM reads ~77 GB/s per stage — far under the DMA roofline.  PE work:
2 stages x FC2*16*2 (or 4*FC2*8*2) matmuls of 512 free = ~1.18M cycles
~ 490 us at 2.4 GHz.
"""

import numpy as np

import concourse.mybir as mybir
import concourse.tile as tile
from concourse import bacc
from concourse.bass_utils import run_bass_kernel_spmd

N_CORES = 8
IN_F = 4096
OUT_F = 4096
NF = 2048
ROWS = 8192
M = ROWS // N_CORES   # 1024 rows per core
P = 128
KH = IN_F // 2        # 2048 folded k' range
KCH = KH // P         # 16 k'-chunks
LH = OUT_F // 2       # 2048 folded l' range
NT = 512
LTH = LH // NT        # 4 l'-tiles
MS = M // P           # 8 row blocks

LAST_RESULTS = None
_NC_CACHE = None


def _build_nc(gcs):
    # gcs: chunks per (a%2, b%2) parity group; group g owns f-chunks
    # [off[g], off[g+1]) and uses xp (g<2) / xm (g>=2) in stage 1,
    # yE (g in {0,2}) / yO (g in {1,3}) in stage 2.
    off = [0]
    for g in range(4):
        off.append(off[-1] + gcs[g])
    FC2 = off[4]
    chunk_grp = []
    for g in range(4):
        chunk_grp += [g] * gcs[g]
    f32 = mybir.dt.float32
    f16 = mybir.dt.float16
    mult = mybir.AluOpType.mult
    add = mybir.AluOpType.add
    sub = mybir.AluOpType.subtract

    nc = bacc.Bacc(None)
    xpT = nc.declare_dram_parameter("xpT", [KH, M], f16, isOutput=False)
    xmT = nc.declare_dram_parameter("xmT", [KH, M], f16, isOutput=False)
    t1c = nc.declare_dram_parameter("t1c", [FC2 * KCH * P, P], f16, isOutput=False)
    t1s = nc.declare_dram_parameter("t1s", [FC2 * KCH * P, P], f16, isOutput=False)
    t2c = nc.declare_dram_parameter("t2c", [FC2 * LTH * P, NT], f16, isOutput=False)
    t2s = nc.declare_dram_parameter("t2s", [FC2 * LTH * P, NT], f16, isOutput=False)
    out = nc.declare_dram_parameter("out", [M, OUT_F], f16, isOutput=True)

    xpp = xpT[:].rearrange("(kc p) m -> p kc m", p=P)
    xmp = xmT[:].rearrange("(kc p) m -> p kc m", p=P)
    t1cp = t1c[:].rearrange("(fb kc p) j -> p fb kc j", fb=FC2, kc=KCH, p=P)
    t1sp = t1s[:].rearrange("(fb kc p) j -> p fb kc j", fb=FC2, kc=KCH, p=P)
    t2cp = t2c[:].rearrange("(fc lt p) l -> p fc lt l", fc=FC2, lt=LTH, p=P)
    t2sp = t2s[:].rearrange("(fc lt p) l -> p fc lt l", fc=FC2, lt=LTH, p=P)
    outp = out[:].rearrange("(ms p) n -> p ms n", p=P)

    with tile.TileContext(nc) as tc:
        with (
            tc.tile_pool(name="v", bufs=1) as vpool,
            tc.tile_pool(name="o", bufs=4) as opool,
        ):
            vc = vpool.tile([P, FC2, M], f16)   # u_cos * 2^-8, 36 KB/part
            vs = vpool.tile([P, FC2, M], f16)

            # ---- stage 1: u^T[f,m] = T1^T @ xfold -> vc/vs
            with (
                tc.tile_pool(name="x", bufs=1) as xpool,
                tc.tile_pool(name="t1", bufs=4) as t1p,
                tc.tile_pool(name="ps1", bufs=4, space="PSUM") as ps1,
            ):
                xp = xpool.tile([P, KCH, M], f16)   # 32 KB/part
                xm = xpool.tile([P, KCH, M], f16)
                # First fb's tables go out first so the PE can start ~4us in;
                # x streams per-kc chunk behind them (xp feeds fb 0.., xm is
                # not needed until the a-odd groups halfway through stage 1).
                pre = []
                for fb in range(2):
                    tcbp = t1p.tile([P, KCH, P], f16, tag="tc")
                    tsbp = t1p.tile([P, KCH, P], f16, tag="ts")
                    nc.sync.dma_start(tcbp[:], t1cp[:, fb, :, :])
                    nc.scalar.dma_start(tsbp[:], t1sp[:, fb, :, :])
                    pre.append((tcbp, tsbp))
                for kc in range(KCH):
                    eng = nc.sync if kc % 2 == 0 else nc.scalar
                    eng.dma_start(xp[:, kc, :], xpp[:, kc, :])
                for kc in range(KCH):
                    eng = nc.sync if kc % 2 == 0 else nc.scalar
                    eng.dma_start(xm[:, kc, :], xmp[:, kc, :])
                for fb in range(FC2):
                    xf = xp if chunk_grp[fb] < 2 else xm
                    if fb < 2:
                        tcb, tsb = pre[fb]
                    else:
                        tcb = t1p.tile([P, KCH, P], f16, tag="tc")
                        tsb = t1p.tile([P, KCH, P], f16, tag="ts")
                        nc.sync.dma_start(tcb[:], t1cp[:, fb, :, :])
                        nc.scalar.dma_start(tsb[:], t1sp[:, fb, :, :])
                    psc = ps1.tile([P, M], f32, tag="u", name=f"psc{fb}")
                    pss = ps1.tile([P, M], f32, tag="u", name=f"pss{fb}")
                    for kc in range(KCH):
                        st, sp = kc == 0, kc == KCH - 1
                        nc.tensor.matmul(psc[:, 0:NT], tcb[:, kc, :],
                                         xf[:, kc, 0:NT], start=st, stop=sp)
                        nc.tensor.matmul(psc[:, NT:M], tcb[:, kc, :],
                                         xf[:, kc, NT:M], start=st, stop=sp)
                        nc.tensor.matmul(pss[:, 0:NT], tsb[:, kc, :],
                                         xf[:, kc, 0:NT], start=st, stop=sp)
                        nc.tensor.matmul(pss[:, NT:M], tsb[:, kc, :],
                                         xf[:, kc, NT:M], start=st, stop=sp)
                    nc.scalar.mul(vc[:, fb, :], psc[:], 2.0 ** -8)
                    nc.vector.tensor_scalar(vs[:, fb, :], pss[:], 2.0 ** -8,
                                            None, mult)

            # ---- stage 2: yE/yO over l' < 2048, y = yE +- yO
            # E accumulates in all 8 PSUM banks, parks in SBUF, then O
            # reuses the banks; combine yE +- psO.  The minus sign of the
            # sin term is folded into t2s on the host.
            e_chunks = [fc for fc in range(FC2) if chunk_grp[fc] % 2 == 0]
            o_chunks = [fc for fc in range(FC2) if chunk_grp[fc] % 2 == 1]
            with (
                tc.tile_pool(name="t2", bufs=8) as t2p,
                tc.tile_pool(name="ye", bufs=1) as yep,
                tc.tile_pool(name="ps2", bufs=8, space="PSUM") as ps2,
            ):
                def run_chunks(chunks, pstag):
                    pss = [ps2.tile([P, NT], f32, tag="y", name=f"{pstag}{ms}")
                           for ms in range(MS)]
                    for fc in chunks:
                        bc = t2p.tile([P, NT], f16, tag="bc")
                        bs = t2p.tile([P, NT], f16, tag="bs")
                        nc.sync.dma_start(bc[:], t2cp[:, fc, lt, :])
                        nc.gpsimd.dma_start(bs[:], t2sp[:, fc, lt, :])
                        for ms in range(MS):
                            nc.tensor.matmul(
                                pss[ms][:], vc[:, fc, ms * P : (ms + 1) * P],
                                bc[:], start=(fc == chunks[0]), stop=False)
                            nc.tensor.matmul(
                                pss[ms][:], vs[:, fc, ms * P : (ms + 1) * P],
                                bs[:], start=False, stop=(fc == chunks[-1]))
                    return pss

                for lt in range(LTH):
                    yE = yep.tile([P, MS, NT], f32, tag="ye", name=f"yE{lt}")
                    psA = run_chunks(e_chunks, f"psA{lt}_")
                    for ms in range(MS):
                        if ms % 2 == 0:
                            nc.scalar.copy(out=yE[:, ms, :], in_=psA[ms][:])
                        else:
                            nc.vector.tensor_copy(out=yE[:, ms, :],
                                                  in_=psA[ms][:])
                    psB = run_chunks(o_chunks, f"psB{lt}_")
                    for ms in range(MS):
                        olo = opool.tile([P, NT], f16, tag="olo", name="olo")
                        ohi = opool.tile([P, NT], f16, tag="ohi", name="ohi")
                        nc.vector.tensor_tensor(out=olo[:], in0=yE[:, ms, :],
                                                in1=psB[ms][:], op=add)
                        nc.vector.tensor_tensor(out=ohi[:], in0=yE[:, ms, :],
                                                in1=psB[ms][:], op=sub)
                        nc.scalar.dma_start(
                            outp[:, ms, lt * NT : (lt + 1) * NT], olo[:])
                        nc.sync.dma_start(
                            outp[:, ms, LH + lt * NT : LH + (lt + 1) * NT],
                            ohi[:])
    nc.finalize()
    return nc


def _host_prep(x, spectrum, indices):
    x2 = np.asarray(x, dtype=np.float32).reshape(ROWS, IN_F)
    idx = np.asarray(indices, dtype=np.int64)
    s = np.asarray(spectrum, dtype=np.float32)
    a, b = idx[0], idx[1]

    # reference scatter is last-write-wins on duplicate (a,b) pairs
    keys = a * OUT_F + b
    _, first_of_reversed = np.unique(keys[::-1], return_index=True)
    keep = np.zeros(NF, dtype=bool)
    keep[NF - 1 - first_of_reversed] = True
    s_eff = np.where(keep, s, 0.0).astype(np.float32)

    # group frequencies by (a%2, b%2); pad each group to a whole number of
    # 128-chunks with zero-spectrum dummies whose indices keep the parity
    sels = [np.nonzero(((a % 2) == (g >> 1)) & ((b % 2) == (g & 1)))[0]
            for g in range(4)]
    gcs = [max(1, -(-len(sel) // P)) for sel in sels]   # chunks per group, >= 1
    FC2 = sum(gcs)
    NF2 = FC2 * P
    a2 = np.zeros(NF2, np.int64)
    b2 = np.zeros(NF2, np.int64)
    s2 = np.zeros(NF2, np.float32)
    o = 0
    for g in range(4):
        sel = sels[g]
        a2[o : o + len(sel)] = a[sel]
        b2[o : o + len(sel)] = b[sel]
        s2[o : o + len(sel)] = s_eff[sel]
        a2[o + len(sel) : o + gcs[g] * P] = g >> 1
        b2[o + len(sel) : o + gcs[g] * P] = g & 1
        o += gcs[g] * P

    w = 2.0 * np.pi / 4096.0
    kk = np.arange(KH)
    ll = np.arange(LH)
    # stage 1 tables [k', f] -> [fb, kc, p(k'), j(f)]
    ph1 = (a2[None, :] * kk[:, None]) % 4096
    t1c_full = np.cos(w * ph1, dtype=np.float32)
    t1s_full = np.sin(w * ph1, dtype=np.float32)
    t1c = np.ascontiguousarray(
        t1c_full.reshape(KCH, P, FC2, P).transpose(2, 0, 1, 3)
        .reshape(FC2 * KCH * P, P).astype(np.float16))
    t1s = np.ascontiguousarray(
        t1s_full.reshape(KCH, P, FC2, P).transpose(2, 0, 1, 3)
        .reshape(FC2 * KCH * P, P).astype(np.float16))
    # stage 2 tables [f, l'] with s*2^-8 folded in (sin term negated so the
    # psum accumulation is a pure add) -> [fc, lt, p(f), l']
    ph2 = (b2[:, None] * ll[None, :]) % 4096
    sc = (s2 * 2.0 ** -8)[:, None]
    t2c_full = np.cos(w * ph2, dtype=np.float32) * sc
    t2s_full = np.sin(w * ph2, dtype=np.float32) * (-sc)
    t2c = np.ascontiguousarray(
        t2c_full.reshape(FC2, P, LTH, NT).transpose(0, 2, 1, 3)
        .reshape(FC2 * LTH * P, NT).astype(np.float16))
    t2s = np.ascontiguousarray(
        t2s_full.reshape(FC2, P, LTH, NT).transpose(0, 2, 1, 3)
        .reshape(FC2 * LTH * P, NT).astype(np.float16))

    xp16 = (x2[:, :KH] + x2[:, KH:]).astype(np.float16)
    xm16 = (x2[:, :KH] - x2[:, KH:]).astype(np.float16)
    return xp16, xm16, t1c, t1s, t2c, t2s, tuple(gcs)


def kernel(x, spectrum, indices):
    global _NC_CACHE, LAST_RESULTS
    xp16, xm16, t1c, t1s, t2c, t2s, gcs = _host_prep(x, spectrum, indices)

    if _NC_CACHE is None or _NC_CACHE[0] != gcs:
        _NC_CACHE = (gcs, _build_nc(gcs))
    nc = _NC_CACHE[1]

    in_maps = [
        {
            "xpT": np.ascontiguousarray(xp16[j * M : (j + 1) * M].T),
            "xmT": np.ascontiguousarray(xm16[j * M : (j + 1) * M].T),
            "t1c": t1c,
            "t1s": t1s,
            "t2c": t2c,
            "t2s": t2s,
        }
        for j in range(N_CORES)
    ]
    res = run_bass_kernel_spmd(nc, in_maps, list(range(N_CORES)))
    LAST_RESULTS = res
    out = np.concatenate(
        [res.results[j]["out"].astype(np.float32) for j in range(N_CORES)], axis=0
    )
    return out.reshape(np.asarray(x).shape[:-1] + (OUT_F,))


# revision 13
# speedup vs baseline: 1.1015x; 1.0765x over previous
"""FourierLinear Trainium2 kernel v5 — host tables + mixed-radix k-folding.

v4 moved all trig-table generation to the host (tables are x-independent
constants streamed from HBM), leaving a pure two-stage fp16 matmul
pipeline.  v5 adds a deeper stage-1 fold for even frequencies:

  stage 1:  u_f[m] = sum_k trig(w a_f k) x[k,m] folded by a_f mod 4:
    a%4==0:  sum_{k'<1024} trig(w a k') (x0+x1+x2+x3)     8 k-chunks
    a%4==2:  sum_{k'<1024} trig(w a k') (x0-x1+x2-x3)     8 k-chunks
    a odd:   sum_{k'<2048} trig(w a k') (x_lo - x_hi)    16 k-chunks
    (odd a gains nothing from mod-4: the quarter terms mix cos<->sin)
  stage 2 (l-parity fold): yE/yO accumulate s_f 2^-8 (uc cos - us sin)
    over even-b / odd-b chunks;  y[l'] = yE+yO, y[l'+2048] = yE-yO

Frequencies are grouped by (fold-type, b%2) into 6 groups, each padded
to whole 128-lane chunks (zero spectrum dummies).  A chunk descriptor
(kch, xsel, bpar) drives both the host table layout and the device
loop.  Stage-1 matmul time is proportional to (number of k-chunks) x m,
so the even-a chunks run at half cost.  The 2^-16 ifft2 norm (* 256) is
split 2^-8 at the stage-1 psum copy and 2^-8 inside the stage-2 tables.
"""

import numpy as np

import concourse.mybir as mybir
import concourse.tile as tile
from concourse import bacc
from concourse.bass_utils import run_bass_kernel_spmd

N_CORES = 8
IN_F = 4096
OUT_F = 4096
NF = 2048
ROWS = 8192
M = ROWS // N_CORES   # 1024 rows per core
P = 128
KH = IN_F // 2        # 2048 folded k' range (odd-a chunks)
KQ = IN_F // 4        # 1024 quarter-folded k' range (even-a chunks)
LH = OUT_F // 2       # 2048 folded l' range
NT = 512
LTH = LH // NT        # 4 l'-tiles
MS = M // P           # 8 row blocks
KCH_MAX = KH // P     # 16

LAST_RESULTS = None
_NC_CACHE = None


def _build_nc(chunks):
    # chunks: tuple of (kch, xsel, bpar) per 128-lane f-chunk.
    #   kch: k-chunks to contract (8 for even-a folds, 16 for odd-a)
    #   xsel: 0 -> xA0 (x0+x1+x2+x3), 1 -> xA2 (x0-x1+x2-x3), 2 -> xm
    #   bpar: b parity (stage-2 yE/yO phase)
    FC2 = len(chunks)
    koff = [0]
    for kch, _, _ in chunks:
        koff.append(koff[-1] + kch)
    TBLK = koff[-1]       # total k-blocks across chunks
    f32 = mybir.dt.float32
    f16 = mybir.dt.float16
    mult = mybir.AluOpType.mult
    add = mybir.AluOpType.add
    sub = mybir.AluOpType.subtract

    nc = bacc.Bacc(None)
    xA0T = nc.declare_dram_parameter("xA0T", [KQ, M], f16, isOutput=False)
    xA2T = nc.declare_dram_parameter("xA2T", [KQ, M], f16, isOutput=False)
    xmT = nc.declare_dram_parameter("xmT", [KH, M], f16, isOutput=False)
    t1c = nc.declare_dram_parameter("t1c", [TBLK * P, P], f16, isOutput=False)
    t1s = nc.declare_dram_parameter("t1s", [TBLK * P, P], f16, isOutput=False)
    t2c = nc.declare_dram_parameter("t2c", [FC2 * LTH * P, NT], f16, isOutput=False)
    t2s = nc.declare_dram_parameter("t2s", [FC2 * LTH * P, NT], f16, isOutput=False)
    out = nc.declare_dram_parameter("out", [M, OUT_F], f16, isOutput=True)

    xA0p = xA0T[:].rearrange("(kc p) m -> p kc m", p=P)
    xA2p = xA2T[:].rearrange("(kc p) m -> p kc m", p=P)
    xmp = xmT[:].rearrange("(kc p) m -> p kc m", p=P)
    t1cp = t1c[:].rearrange("(blk p) j -> p blk j", p=P)
    t1sp = t1s[:].rearrange("(blk p) j -> p blk j", p=P)
    t2cp = t2c[:].rearrange("(fc lt p) l -> p fc lt l", fc=FC2, lt=LTH, p=P)
    t2sp = t2s[:].rearrange("(fc lt p) l -> p fc lt l", fc=FC2, lt=LTH, p=P)
    outp = out[:].rearrange("(ms p) n -> p ms n", p=P)

    with tile.TileContext(nc) as tc:
        with (
            tc.tile_pool(name="v", bufs=1) as vpool,
            tc.tile_pool(name="o", bufs=4) as opool,
        ):
            vc = vpool.tile([P, FC2, M], f16)   # u_cos * 2^-8
            vs = vpool.tile([P, FC2, M], f16)

            # ---- stage 1
            with (
                tc.tile_pool(name="x", bufs=1) as xpool,
                tc.tile_pool(name="t1", bufs=4) as t1p,
                tc.tile_pool(name="ps1", bufs=4, space="PSUM") as ps1,
            ):
                xA0 = xpool.tile([P, KQ // P, M], f16)   # 16 KB/part
                xA2 = xpool.tile([P, KQ // P, M], f16)
                xm = xpool.tile([P, KCH_MAX, M], f16)    # 32 KB/part
                xtiles = [xA0, xA2, xm]
                xaps = [xA0p, xA2p, xmp]

                def tbl_tiles(i):
                    kch = chunks[i][0]
                    tcb = t1p.tile([P, KCH_MAX, P], f16, tag="tc")
                    tsb = t1p.tile([P, KCH_MAX, P], f16, tag="ts")
                    nc.sync.dma_start(
                        tcb[:, 0:kch, :], t1cp[:, koff[i] : koff[i] + kch, :])
                    nc.scalar.dma_start(
                        tsb[:, 0:kch, :], t1sp[:, koff[i] : koff[i] + kch, :])
                    return tcb, tsb

                # prefetch first two chunks' tables, then stream x behind
                # them: the first chunk only needs its x-fold piecewise.
                pre = [tbl_tiles(i) for i in range(2)]
                # x order: the fold the first chunks use goes first
                first_xsel = chunks[0][1]
                xorder = [first_xsel] + [i for i in range(3) if i != first_xsel]
                xkch = [KQ // P, KQ // P, KCH_MAX]
                qi = 0
                for xi in xorder:
                    xt, xa = xtiles[xi], xaps[xi]
                    for kc in range(xkch[xi]):
                        eng = nc.sync if qi % 2 == 0 else nc.scalar
                        eng.dma_start(xt[:, kc, :], xa[:, kc, :])
                        qi += 1

                for i, (kch, xsel, _) in enumerate(chunks):
                    xf = xtiles[xsel]
                    tcb, tsb = pre[i] if i < 2 else tbl_tiles(i)
                    psc = ps1.tile([P, M], f32, tag="u", name=f"psc{i}")
                    pss = ps1.tile([P, M], f32, tag="u", name=f"pss{i}")
                    for kc in range(kch):
                        st, sp = kc == 0, kc == kch - 1
                        nc.tensor.matmul(psc[:, 0:NT], tcb[:, kc, :],
                                         xf[:, kc, 0:NT], start=st, stop=sp)
                        nc.tensor.matmul(psc[:, NT:M], tcb[:, kc, :],
                                         xf[:, kc, NT:M], start=st, stop=sp)
                        nc.tensor.matmul(pss[:, 0:NT], tsb[:, kc, :],
                                         xf[:, kc, 0:NT], start=st, stop=sp)
                        nc.tensor.matmul(pss[:, NT:M], tsb[:, kc, :],
                                         xf[:, kc, NT:M], start=st, stop=sp)
                    nc.scalar.mul(vc[:, i, :], psc[:], 2.0 ** -8)
                    nc.vector.tensor_scalar(vs[:, i, :], pss[:], 2.0 ** -8,
                                            None, mult)

            # ---- stage 2: yE/yO over l' < 2048, y = yE +- yO
            e_chunks = [i for i in range(FC2) if chunks[i][2] == 0]
            o_chunks = [i for i in range(FC2) if chunks[i][2] == 1]
            with (
                tc.tile_pool(name="t2", bufs=8) as t2p,
                tc.tile_pool(name="ye", bufs=1) as yep,
                tc.tile_pool(name="ps2", bufs=8, space="PSUM") as ps2,
            ):
                def run_chunks(chunk_ids, pstag):
                    pss = [ps2.tile([P, NT], f32, tag="y", name=f"{pstag}{ms}")
                           for ms in range(MS)]
                    for fc in chunk_ids:
                        bc = t2p.tile([P, NT], f16, tag="bc")
                        bs = t2p.tile([P, NT], f16, tag="bs")
                        nc.sync.dma_start(bc[:], t2cp[:, fc, lt, :])
                        nc.gpsimd.dma_start(bs[:], t2sp[:, fc, lt, :])
                        for ms in range(MS):
                            nc.tensor.matmul(
                                pss[ms][:], vc[:, fc, ms * P : (ms + 1) * P],
                                bc[:], start=(fc == chunk_ids[0]), stop=False)
                            nc.tensor.matmul(
                                pss[ms][:], vs[:, fc, ms * P : (ms + 1) * P],
                                bs[:], start=False,
                                stop=(fc == chunk_ids[-1]))
                    return pss

                for lt in range(LTH):
                    yE = yep.tile([P, MS, NT], f32, tag="ye", name=f"yE{lt}")
                    psA = run_chunks(e_chunks, f"psA{lt}_")
                    for ms in range(MS):
                        if ms % 2 == 0:
                            nc.scalar.copy(out=yE[:, ms, :], in_=psA[ms][:])
                        else:
                            nc.vector.tensor_copy(out=yE[:, ms, :],
                                                  in_=psA[ms][:])
                    psB = run_chunks(o_chunks, f"psB{lt}_")
                    for ms in range(MS):
                        olo = opool.tile([P, NT], f16, tag="olo", name="olo")
                        ohi = opool.tile([P, NT], f16, tag="ohi", name="ohi")
                        nc.vector.tensor_tensor(out=olo[:], in0=yE[:, ms, :],
                                                in1=psB[ms][:], op=add)
                        nc.vector.tensor_tensor(out=ohi[:], in0=yE[:, ms, :],
                                                in1=psB[ms][:], op=sub)
                        nc.scalar.dma_start(
                            outp[:, ms, lt * NT : (lt + 1) * NT], olo[:])
                        nc.sync.dma_start(
                            outp[:, ms, LH + lt * NT : LH + (lt + 1) * NT],
                            ohi[:])
    nc.finalize()
    return nc


# (group selector, fold kch, xsel, dummy a) for the 6 (fold-type, b-par)
# groups; b parity is appended per group below.
_GROUPS = [
    (lambda a: a % 4 == 0, 8, 0, 0),
    (lambda a: a % 4 == 2, 8, 1, 2),
    (lambda a: a % 2 == 1, 16, 2, 1),
]


def _host_prep(x, spectrum, indices):
    x2 = np.asarray(x, dtype=np.float32).reshape(ROWS, IN_F)
    idx = np.asarray(indices, dtype=np.int64)
    s = np.asarray(spectrum, dtype=np.float32)
    a, b = idx[0], idx[1]

    # reference scatter is last-write-wins on duplicate (a,b) pairs
    keys = a * OUT_F + b
    _, first_of_reversed = np.unique(keys[::-1], return_index=True)
    keep = np.zeros(NF, dtype=bool)
    keep[NF - 1 - first_of_reversed] = True
    s_eff = np.where(keep, s, 0.0).astype(np.float32)

    # build padded per-chunk lane arrays + chunk descriptors
    lanes_a, lanes_b, lanes_s, chunks = [], [], [], []
    for bpar in (0, 1):
        for selfn, kch, xsel, dummy_a in _GROUPS:
            sel = np.nonzero(selfn(a) & ((b % 2) == bpar))[0]
            n = max(1, -(-len(sel) // P)) * P
            ga = np.full(n, dummy_a, np.int64)
            gb = np.full(n, bpar, np.int64)
            gs = np.zeros(n, np.float32)
            ga[: len(sel)] = a[sel]
            gb[: len(sel)] = b[sel]
            gs[: len(sel)] = s_eff[sel]
            lanes_a.append(ga)
            lanes_b.append(gb)
            lanes_s.append(gs)
            chunks += [(kch, xsel, bpar)] * (n // P)
    a2 = np.concatenate(lanes_a)
    b2 = np.concatenate(lanes_b)
    s2 = np.concatenate(lanes_s)
    FC2 = len(chunks)

    w = 2.0 * np.pi / 4096.0
    # stage-1 tables, flat [sum kch*P rows, 128 lanes] per chunk
    t1c_parts, t1s_parts = [], []
    for i, (kch, _, _) in enumerate(chunks):
        al = a2[i * P : (i + 1) * P]
        kq = np.arange(kch * P)
        ph = (al[None, :] * kq[:, None]) % 4096
        t1c_parts.append(np.cos(w * ph, dtype=np.float32))
        t1s_parts.append(np.sin(w * ph, dtype=np.float32))
    t1c = np.ascontiguousarray(
        np.concatenate(t1c_parts, axis=0).astype(np.float16))
    t1s = np.ascontiguousarray(
        np.concatenate(t1s_parts, axis=0).astype(np.float16))

    # stage-2 tables [f, l'] with s*2^-8 folded in (sin negated: pure adds)
    ll = np.arange(LH)
    ph2 = (b2[:, None] * ll[None, :]) % 4096
    sc = (s2 * 2.0 ** -8)[:, None]
    t2c_full = np.cos(w * ph2, dtype=np.float32) * sc
    t2s_full = np.sin(w * ph2, dtype=np.float32) * (-sc)
    t2c = np.ascontiguousarray(
        t2c_full.reshape(FC2, P, LTH, NT).transpose(0, 2, 1, 3)
        .reshape(FC2 * LTH * P, NT).astype(np.float16))
    t2s = np.ascontiguousarray(
        t2s_full.reshape(FC2, P, LTH, NT).transpose(0, 2, 1, 3)
        .reshape(FC2 * LTH * P, NT).astype(np.float16))

    q0, q1, q2, q3 = (x2[:, i * KQ : (i + 1) * KQ] for i in range(4))
    xA0 = (q0 + q1 + q2 + q3).astype(np.float16)
    xA2 = (q0 - q1 + q2 - q3).astype(np.float16)
    xm16 = (x2[:, :KH] - x2[:, KH:]).astype(np.float16)
    return xA0, xA2, xm16, t1c, t1s, t2c, t2s, tuple(chunks)


def kernel(x, spectrum, indices):
    global _NC_CACHE, LAST_RESULTS
    xA0, xA2, xm16, t1c, t1s, t2c, t2s, chunks = _host_prep(x, spectrum, indices)

    if _NC_CACHE is None or _NC_CACHE[0] != chunks:
        _NC_CACHE = (chunks, _build_nc(chunks))
    nc = _NC_CACHE[1]

    in_maps = [
        {
            "xA0T": np.ascontiguousarray(xA0[j * M : (j + 1) * M].T),
            "xA2T": np.ascontiguousarray(xA2[j * M : (j + 1) * M].T),
            "xmT": np.ascontiguousarray(xm16[j * M : (j + 1) * M].T),
            "t1c": t1c,
            "t1s": t1s,
            "t2c": t2c,
            "t2s": t2s,
        }
        for j in range(N_CORES)
    ]
    res = run_bass_kernel_spmd(nc, in_maps, list(range(N_CORES)))
    LAST_RESULTS = res
    out = np.concatenate(
        [res.results[j]["out"].astype(np.float32) for j in range(N_CORES)], axis=0
    )
    return out.reshape(np.asarray(x).shape[:-1] + (OUT_F,))
